# revision 34
# baseline (speedup 1.0000x reference)
# Trainium2 Bass kernel for nn_LAB_42906723287350.
#
#   probs = softmax(choice_parameters, axis=0); s = x @ probs
#   out = mix0*multilinear(sigmoid(lut); s) + mix1*clip(s0+s1+s2-2,0,1)
#         + mix2*(s0+s1+s2>=2)
#
# Data parallel over 8 cores (R=131072 rows each).  Row mapping per core:
#   row = ROWS_TILE*nd + T_IL*p + t   (nd = tile, p = partition, t in [0,T_IL))
# Per tile nd: SWDGE cast-DMA x [128, (t,c)] f32->bf16 (8KB/partition reads)
#   -> T_IL/2 PE transposes (bf16) -> xT [(t2,c), p] psum bf16 -> copy
#   -> T_IL/2 accumulating matmuls with xT as STATIONARY and P [128, 4*T_IL]
#      moving -> s in NATURAL layout [128 p, (j,t)] (no back-transpose).
# Per group (sizes [8,8,8,4,4]): one f32 copy sn PSUM -> SBUF (fast PSUM
#   recycle), 16-coeff Horner poly + add path in f32 (leaves on ACT, tree on
#   DVE; the final small group is split across DVE and Pool to halve the
#   tail chain) -> vo2 f32 [128, (k,t)] -> ONE strided HWDGE DMA straight to
#   out (per partition: aligned 128B chunks).  No back-transposes.
import numpy as np
import ml_dtypes

import concourse.bass as bass
import concourse.mybir as mybir
import concourse.tile as tile
from concourse import bacc
from concourse.bass_utils import run_bass_kernel_spmd

N_CORES = 8
B_FULL = 1048576
CIN = 64
F32 = mybir.dt.float32
MM_DT = mybir.dt.bfloat16
ALU = mybir.AluOpType
AF = mybir.ActivationFunctionType

T_IL = 32                        # rows interleaved per partition
H_BLK = T_IL // 2                # 128-partition transpose sub-blocks per tile
XH = min(H_BLK, 8)               # sub-blocks per xT PSUM tile (1 bank)
ROWS_TILE = 128 * T_IL           # 4096 rows per x-tile
JT = 4 * T_IL                    # s-matmul output cols per tile (j,t)
PREF = 5                         # x DMAs issued before const setup


def group_sizes(n_tiles):
    # groups of 8, with the final 8 split into 4+4 to shorten the tail
    assert n_tiles % 8 == 0 and n_tiles >= 16
    return [8] * (n_tiles // 8 - 1) + [4, 4]


def build_nc(R, mm_dtype=MM_DT):
    n_tiles = R // ROWS_TILE
    groups = group_sizes(n_tiles)

    nc = bacc.Bacc()
    x_d = nc.dram_tensor("x", [R, CIN], F32, kind="ExternalInput")
    pm_d = nc.dram_tensor("pm", [128, H_BLK * JT], mm_dtype, kind="ExternalInput")
    coef_d = nc.dram_tensor("coef", [128, 19], F32, kind="ExternalInput")
    ident_d = nc.dram_tensor("ident", [128, 128], mm_dtype, kind="ExternalInput")
    out_d = nc.dram_tensor("out", [R, 1], F32, kind="ExternalOutput")

    # x row = ROWS_TILE*nd + T_IL*p + t ; tile view [nd, p, (t c)]
    x2v = x_d[:].rearrange("(nd p t) c -> nd p (t c)", p=128, t=T_IL)

    with tile.TileContext(nc) as tc:
        with (
            tc.tile_pool(name="const", bufs=1) as cpool,
            tc.tile_pool(name="xin", bufs=13) as xpool,
            tc.tile_pool(name="xtsb", bufs=6) as xsbpool,
            tc.tile_pool(name="sbf", bufs=2) as sbfpool,
            tc.tile_pool(name="tmp", bufs=2) as tpool,
            tc.tile_pool(name="vo", bufs=3) as vopool,
            tc.tile_pool(name="psxt", bufs=2, space="PSUM") as ppxt,
            tc.tile_pool(name="pssn", bufs=2, space="PSUM") as ppsn,
        ):
            # issue the first x-tile DMAs before anything else so SDMA ramps
            # while the constants/identity get set up.  Tile 0 is fetched in
            # two halves so its first transposes can start earlier.
            x_tiles = {}
            xt0 = xpool.tile([128, 128 * H_BLK], mm_dtype, tag="x")
            half = 128 * H_BLK // 2
            nc.gpsimd.dma_start(out=xt0[:, :half], in_=x2v[0][:, :half])
            nc.gpsimd.dma_start(out=xt0[:, half:], in_=x2v[0][:, half:])
            x_tiles[0] = xt0
            for nd in range(1, min(PREF, n_tiles)):
                xt = xpool.tile([128, 128 * H_BLK], mm_dtype, tag="x")
                nc.gpsimd.dma_start(out=xt[:], in_=x2v[nd])
                x_tiles[nd] = xt

            identb = cpool.tile([128, 128], mm_dtype)
            nc.sync.dma_start(out=identb[:], in_=ident_d[:])
            pm_sb = cpool.tile([128, H_BLK * JT], mm_dtype)
            nc.sync.dma_start(out=pm_sb[:], in_=pm_d[:])
            coef_sb = cpool.tile([128, 19], F32)
            nc.sync.dma_start(out=coef_sb[:], in_=coef_d[:])

            def poly_ops(s_bf, vo2, gs, split_leaves):
                # Returns a list of thunks (one engine op each) computing the
                # poly for a staged group.  Emitted lazily so the ops can be
                # interleaved into the NEXT group's per-tile stream (keeps
                # the DVE queue from blocking that group's xT copies).
                sv = s_bf[:].rearrange(
                    "p (k j t) -> p k j t", k=gs, j=4, t=T_IL
                )
                s = [sv[:, :, jj, :] for jj in range(4)]

                def tmp(tag):
                    tl = tpool.tile([128, gs * T_IL], F32, tag=f"{tag}{gs}")
                    return tl[:].rearrange(
                        "p (k t) -> p k t", k=gs, t=T_IL
                    )

                ops = []
                # leaves L_i = coef[8+i]*s0 + coef[i] on ACT (split with DVE
                # on the tail group so the two engines chew them in parallel)
                Lf = [tmp(f"leaf{i}") for i in range(8)]
                for i in range(8):
                    v, ii = Lf[i], i
                    if split_leaves and i % 2 == 1:
                        ops.append(lambda v=v, ii=ii: nc.vector.tensor_scalar(
                            v, s[0], coef_sb[:, 8 + ii : 9 + ii],
                            coef_sb[:, ii : ii + 1], op0=ALU.mult, op1=ALU.add,
                        ))
                    else:
                        ops.append(lambda v=v, ii=ii: nc.scalar.activation(
                            v, s[0], AF.Identity,
                            bias=coef_sb[:, ii : ii + 1],
                            scale=coef_sb[:, 8 + ii : 9 + ii],
                        ))
                G = [tmp(f"gm{k}") for k in range(4)]
                for k in range(4):
                    v = G[k]
                    ops.append(lambda v=v, k=k: nc.vector.tensor_mul(
                        v, s[1], Lf[2 * k + 1]))
                    ops.append(lambda v=v, k=k: nc.vector.tensor_add(
                        v, v, Lf[2 * k]))
                H = [tmp(f"hm{m}") for m in range(2)]
                for m in range(2):
                    v = H[m]
                    ops.append(lambda v=v, m=m: nc.vector.tensor_mul(
                        v, s[2], G[2 * m + 1]))
                    ops.append(lambda v=v, m=m: nc.vector.tensor_add(
                        v, v, G[2 * m]))
                vl = tmp("lut")
                ops.append(lambda: nc.vector.tensor_mul(vl, s[3], H[1]))
                ops.append(lambda: nc.vector.tensor_add(vl, vl, H[0]))
                va = tmp("adds")
                ops.append(lambda: nc.vector.tensor_add(va, s[0], s[1]))
                ops.append(lambda: nc.vector.tensor_add(va, va, s[2]))
                vr = tmp("relu")
                ops.append(lambda: nc.scalar.activation(
                    vr, va, AF.Relu, bias=coef_sb[:, 18:19], scale=1.0))
                ops.append(lambda: nc.vector.tensor_scalar(
                    vr, vr, 1.0, coef_sb[:, 16:17], op0=ALU.min, op1=ALU.mult))
                vq = tmp("step")
                ops.append(lambda: nc.vector.tensor_scalar(
                    vq, va, 2.0, coef_sb[:, 17:18], op0=ALU.is_ge, op1=ALU.mult))
                vo2v = vo2[:].rearrange("p (k t) -> p k t", k=gs, t=T_IL)
                ops.append(lambda: nc.vector.tensor_add(vo2v, vl, vr))
                ops.append(lambda: nc.vector.tensor_add(vo2v, vo2v, vq))
                return ops

            def store_dma(t0, gs, vo2, eng):
                # direct strided store: per partition gs aligned 128B chunks.
                # Non-final stores go via SWDGE (gpsimd): their descriptors
                # land in the q0 rings BEHIND all x-load descriptors, so
                # they execute in the post-load idle window instead of
                # stealing x bandwidth.  The final store uses HWDGE (sync)
                # for its lower descriptor-generation latency in the tail.
                ov = out_d[ROWS_TILE * t0 : ROWS_TILE * (t0 + gs)].rearrange(
                    "(k p t) one -> p k (t one)", k=gs, p=128, t=T_IL
                )
                eng.dma_start(
                    out=ov,
                    in_=vo2[:].rearrange("p (k t) -> p k t", k=gs, t=T_IL),
                )

            pending = []  # deferred (thunk) ops: prev group's poly
            late_stores = []  # non-final group stores, run during the tail

            def drain(n):
                for _ in range(min(n, len(pending))):
                    pending.pop(0)()

            t0 = 0
            xt_final = None
            for gi, gs in enumerate(groups):
                last = gi == len(groups) - 1
                vo2 = vopool.tile([128, gs * T_IL], F32, tag=f"vo2{gs}")
                sn_ps = ppsn.tile([128, JT * gs], F32, tag=f"sn{gs}")
                for kk in range(gs):
                    nd = t0 + kk
                    xt = x_tiles.pop(nd)
                    if nd == n_tiles - 1:
                        xt_final = xt
                    xT_sb = xsbpool.tile([128, 128 * H_BLK], mm_dtype, tag="xTsb")
                    for hh in range(H_BLK // XH):
                        xT_ps = ppxt.tile([128, 128 * XH], mm_dtype, tag="xT")
                        for h2 in range(XH):
                            h = XH * hh + h2
                            nc.tensor.transpose(
                                xT_ps[:, 128 * h2 : 128 * (h2 + 1)],
                                xt[:, 128 * h : 128 * (h + 1)],
                                identb[:],
                            )
                        dst = xT_sb[:, 128 * XH * hh : 128 * XH * (hh + 1)]
                        if (nd + hh) % 2 == 0:
                            nc.scalar.copy(out=dst, in_=xT_ps[:])
                        else:
                            nc.vector.tensor_copy(out=dst, in_=xT_ps[:])
                    # keep the x DMA queue primed PREF tiles ahead
                    nxt = nd + PREF
                    if nxt < n_tiles:
                        xtn = xpool.tile(
                            [128, 128 * H_BLK], mm_dtype, tag="x"
                        )
                        if nxt == n_tiles - 1:
                            # split the final tile so its transposes can
                            # start on the first half
                            nc.gpsimd.dma_start(
                                out=xtn[:, :half], in_=x2v[nxt][:, :half]
                            )
                            nc.gpsimd.dma_start(
                                out=xtn[:, half:], in_=x2v[nxt][:, half:]
                            )
                        else:
                            nc.gpsimd.dma_start(out=xtn[:], in_=x2v[nxt])
                        x_tiles[nxt] = xtn
                    for h in range(H_BLK):
                        nc.tensor.matmul(
                            sn_ps[:, JT * kk : JT * (kk + 1)],
                            lhsT=xT_sb[:, 128 * h : 128 * (h + 1)],
                            rhs=pm_sb[:, JT * h : JT * (h + 1)],
                            start=(h == 0),
                            stop=(h == H_BLK - 1),
                        )
                    # interleave the previous group's deferred poly ops so
                    # they never queue ahead of this group's copies on DVE
                    drain(4)
                drain(len(pending))
                # stage s out of PSUM fast (f32, 2-port copy), then poly
                s_bf = sbfpool.tile([128, JT * gs], F32, tag=f"sbf{gs}")
                if gi % 2 == 0:
                    nc.scalar.copy(out=s_bf[:], in_=sn_ps[:])
                else:
                    nc.vector.tensor_copy(out=s_bf[:], in_=sn_ps[:])
                pending.extend(poly_ops(s_bf, vo2, gs, split_leaves=last))
                if last:
                    for st in late_stores:
                        st()
                    drain(len(pending))
                    store_dma(t0, gs, vo2, nc.sync)
                else:
                    late_stores.append(
                        lambda t0=t0, gs=gs, vo2=vo2: store_dma(
                            t0, gs, vo2, nc.gpsimd
                        )
                    )
                t0 += gs
    nc.compile()
    return nc


def host_prep(choice_parameters, lut, lut_vs_add_choice_parameters, mm_np=ml_dtypes.bfloat16):
    cp = np.asarray(choice_parameters, dtype=np.float64)
    e = np.exp(cp - cp.max(axis=0, keepdims=True))
    probs = e / e.sum(axis=0, keepdims=True)  # [64,4]
    L = 1.0 / (1.0 + np.exp(-np.asarray(lut, dtype=np.float64)))
    m = np.asarray(lut_vs_add_choice_parameters, dtype=np.float64)
    em = np.exp(m - m.max())
    mix = em / em.sum()

    c = np.zeros(16)
    for Sm in range(16):
        v = L
        for ax in range(4):
            vec = np.array([1.0, -1.0]) if (Sm >> ax) & 1 else np.array([0.0, 1.0])
            v = np.tensordot(v, vec, axes=([0], [0]))
        c[Sm] = float(v) * mix[0]

    coef_row = np.zeros(19)
    for idx in range(8):
        coef_row[idx] = c[idx << 1]
        coef_row[8 + idx] = c[(idx << 1) | 1]
    coef_row[16] = mix[1]
    coef_row[17] = mix[2]
    coef_row[18] = -2.0
    coef = np.tile(coef_row.astype(np.float32)[None], (128, 1))

    # pm[t2*64+c, JT*h + T_IL*j + t] = probs[c,j] * [t == 2h+t2]
    pm = np.zeros((128, H_BLK * JT), np.float64)
    for h in range(H_BLK):
        for t2 in range(2):
            for cc in range(64):
                for j in range(4):
                    pm[t2 * 64 + cc, JT * h + T_IL * j + (2 * h + t2)] = probs[cc, j]
    pm = pm.astype(mm_np)
    ident = np.eye(128, dtype=mm_np)
    return pm, coef, ident


_NC_CACHE = {}


def _get_nc(R):
    if R not in _NC_CACHE:
        _NC_CACHE[R] = build_nc(R)
    return _NC_CACHE[R]


def run_on_hw(x, choice_parameters, lut, lut_vs_add_choice_parameters, **kw):
    x = np.ascontiguousarray(np.asarray(x, dtype=np.float32))
    R = x.shape[0] // N_CORES
    nc = _get_nc(R)
    pm, coef, ident = host_prep(choice_parameters, lut, lut_vs_add_choice_parameters)
    in_maps = [
        {"x": np.ascontiguousarray(x[i * R : (i + 1) * R]), "pm": pm,
         "coef": coef, "ident": ident}
        for i in range(N_CORES)
    ]
    res = run_bass_kernel_spmd(nc, in_maps, list(range(N_CORES)), **kw)
    out = np.concatenate([r["out"] for r in res.results], axis=0)
    return out, res


def kernel(x, choice_parameters, lut, lut_vs_add_choice_parameters):
    out, _ = run_on_hw(x, choice_parameters, lut, lut_vs_add_choice_parameters)
    return out
